# revision 1
# baseline (speedup 1.0000x reference)
"""Trainium2 Bass kernel for nn_CrossAttentionBlock (B=2, S=2048, D=1024, H=16, HD=64).

Sharding: 8 cores = 2 batches x 4 head-quads (4 heads each, E=256 channels).
Each core computes q/k/v projections for its quad, RoPE, SDPA, and a partial
output projection [S, D]; host sums the 4 partials per batch and adds bo.

Device pipeline (all matmul operands bf16, fp32 PSUM accumulation):
  - host ships x^T and W^T (d-major) with a ones-row appended to x^T and the
    bias as an extra weight row, so biases are exact.
  - RoPE is GPT-NeoX-interleaved; we fold the even/odd channel permutation into
    the q/k weight rows on the host (dot products are permutation invariant),
    which turns it into rot-half RoPE: out = q*cos + swap_halves(q)*sin with
    sign folded into the sin table. swap_halves is a partition swap done by
    SBUF->SBUF DMA; cos/sin tables are elementwise DVE multiplies.
  - scores^T [k_, q] per head via row-tiled PE pairs (k=64 each, concurrent),
    exp on ACT (PSUM->SBUF bf16, FD=1024 per head-pair), ctx^T accumulation via
    col-tiled PE pairs, softmax denominators via ones-matmul m=1 column groups.
  - normalization of ctx by 1/l (reciprocal_approx_fast + DMA partition
    broadcast) happens before the output projection so the per-head scale is
    applied before heads are mixed.
"""
import os
import sys

sys.path.insert(0, "/opt/trn_rl_repo")

import numpy as np
import ml_dtypes

BF16 = ml_dtypes.bfloat16

B, S, D, H = 2, 2048, 1024, 16
HD = D // H          # 64
DIM = HD // 2        # 32
QUADS = 4            # head groups of 4
E = D // QUADS       # 256 channels per core
ROPE_BASE = 10000.0
N_CORES = 8

KTILES = D // 128    # 8  (plus one bias row)
ST = S // 128        # 16 s-tiles
QC = S // 512        # 4 q-chunks


def _host_prep(x_q, x_kv, wq, bq, wk, bk, wv, bv, wo):
    """Build the per-core input maps (all bf16 except noted)."""
    perm = np.concatenate([np.arange(0, HD, 2), np.arange(1, HD, 2)])  # even|odd
    scale = 1.0 / np.sqrt(HD)

    freqs = np.exp(-np.arange(DIM, dtype=np.float64) * np.log(ROPE_BASE) / DIM)
    ang = np.arange(S, dtype=np.float64)[:, None] * freqs[None, :]     # [S, 32]
    cos = np.cos(ang).T                                                # [32, S]
    sin = np.sin(ang).T
    # [e(64), s] tables for one head-block, repeated across the two heads of a
    # 128-partition pair tile.  rot-half: out = q*cos + swap(q)*sin_signed
    cos64 = np.concatenate([cos, cos], axis=0)                         # [64, S]
    sin64 = np.concatenate([-sin, sin], axis=0)
    cosT = np.concatenate([cos64, cos64], axis=0).astype(BF16)         # [128, S]
    sinT = np.concatenate([sin64, sin64], axis=0).astype(BF16)

    def proj_mat(w, b, permute, s):
        # rows for one quad stacked [256, 1024] (+bias row) -> [1025, 256] d-major
        blocks, brows = [], []
        for h in range(4):
            rows = slice(h * HD, (h + 1) * HD)
            wb = w[rows, :]
            bb = b[rows]
            if permute:
                wb = wb[perm, :]
                bb = bb[perm]
            blocks.append(wb * s)
            brows.append(bb * s)
        wstack = np.concatenate(blocks, axis=0)          # [256, 1024]
        bstack = np.concatenate(brows, axis=0)           # [256]
        return np.concatenate([wstack.T, bstack[None, :]], axis=0)  # [1025, 256]

    ones_row = np.ones((1, S), dtype=np.float32)
    in_maps = []
    for c in range(N_CORES):
        b_ = c // QUADS
        g = c % QUADS
        hs = slice(g * E, (g + 1) * E)  # channel rows of this quad
        xqT = np.concatenate([x_q[b_].T, ones_row], axis=0).astype(BF16)    # [1025,S]
        xkvT = np.concatenate([x_kv[b_].T, ones_row], axis=0).astype(BF16)
        wq_g = np.ascontiguousarray(
            proj_mat(wq[hs, :], bq[hs], True, scale)).astype(BF16)
        wk_g = np.ascontiguousarray(
            proj_mat(wk[hs, :], bk[hs], True, 1.0)).astype(BF16)
        wv_g = np.ascontiguousarray(
            proj_mat(wv[hs, :], bv[hs], False, 1.0)).astype(BF16)
        woT_g = np.ascontiguousarray(wo[:, hs].T).astype(BF16)             # [256,1024]
        in_maps.append({
            "xqT": xqT, "xkvT": xkvT,
            "wqT": wq_g, "wkT": wk_g, "wvT": wv_g, "woT": woT_g,
            "cosT": np.ascontiguousarray(cosT),
            "sinT": np.ascontiguousarray(sinT),
            "ones_col": np.ones((128, 1), dtype=BF16),
        })
    return in_maps


# ---------------------------------------------------------------------------
_PROGRAM_CACHE = {}


def _fixed_tile_context(tile_mod, bass_rust_mod, vector_clock_mod):
    """TileContext whose tail drain splits multi-sem waits into single-wait
    NOPs (this walrus rejects >1 sync-wait on one instruction)."""
    SyncInfo = bass_rust_mod.SyncInfo
    ScopedClock = vector_clock_mod.ScopedClock

    class TC(tile_mod.TileContext):
        def _drain_and_barrier(self, tick_clock, wait_clock):
            harvest = self.nc.sync.nop(nofuse=True)
            wait_clock.add_sem_waits(
                harvest.ins, ScopedClock({None: tick_clock.global_clock}))
            si = harvest.ins.sync_info
            waits = list(si.on_wait) if si is not None else []
            if len(waits) > 1:
                harvest.ins.sync_info = SyncInfo(
                    on_wait=[waits[0]], on_update=list(si.on_update))
                for w in waits[1:]:
                    nop = self.nc.sync.nop(nofuse=True)
                    nop.ins.sync_info = SyncInfo(on_wait=[w], on_update=[])
            self.nc.sync.drain()
            self.nc.all_engine_barrier()
            assert self.sems is not None
            popped = self.nc._tile_sem_poison_stack.pop()
            assert popped is self._sem_poison
            self.nc.clear_and_free_semaphores(list(self.sems.allocated().values()))
            self.nc.all_engine_barrier()

    return TC


def _split_multiwait_instructions(nc, mybir, SyncInfo):
    """This walrus build rejects >1 sync-wait per instruction; hoist extra
    waits onto single-wait NOPs inserted just before, on the same engine."""
    ctr = 0
    for blk in nc.m.functions[0].blocks:
        insts = blk.instructions
        i = 0
        while i < len(insts):
            inst = insts[i]
            si = inst.sync_info
            if si is not None and len(si.on_wait) > 1:
                waits = list(si.on_wait)
                inst.sync_info = SyncInfo(on_wait=[waits[-1]],
                                          on_update=list(si.on_update))
                nops = []
                for w in waits[:-1]:
                    nop = mybir.InstNoOp(name=f"waitsplit_{ctr}", ins=[], outs=[])
                    ctr += 1
                    nop.engine = inst.engine
                    nop.sync_info = SyncInfo(on_wait=[w], on_update=[])
                    nops.append(nop)
                insts[i:i] = nops
                i += len(nops)
            i += 1
    return ctr


def build_program(split_waits=True):
    import concourse.bass as bass
    import concourse.mybir as mybir
    import concourse.tile as tile
    import bass_rust
    from concourse import vector_clock
    from concourse import library_config

    f32 = mybir.dt.float32
    bf16 = mybir.dt.bfloat16
    Exp = mybir.ActivationFunctionType.Exp
    Copy = mybir.ActivationFunctionType.Copy
    mult = mybir.AluOpType.mult
    add = mybir.AluOpType.add

    nc = bass.Bass("TRN2", target_bir_lowering=False, debug=False,
                   num_devices=N_CORES)

    xqT = nc.dram_tensor("xqT", [D + 1, S], bf16, kind="ExternalInput").ap()
    xkvT = nc.dram_tensor("xkvT", [D + 1, S], bf16, kind="ExternalInput").ap()
    wqT = nc.dram_tensor("wqT", [D + 1, E], bf16, kind="ExternalInput").ap()
    wkT = nc.dram_tensor("wkT", [D + 1, E], bf16, kind="ExternalInput").ap()
    wvT = nc.dram_tensor("wvT", [D + 1, E], bf16, kind="ExternalInput").ap()
    woT = nc.dram_tensor("woT", [E, D], bf16, kind="ExternalInput").ap()
    cosT = nc.dram_tensor("cosT", [128, S], bf16, kind="ExternalInput").ap()
    sinT = nc.dram_tensor("sinT", [128, S], bf16, kind="ExternalInput").ap()
    ones_col = nc.dram_tensor("ones_col", [128, 1], bf16, kind="ExternalInput").ap()
    out = nc.dram_tensor("out", [S, D], f32, kind="ExternalOutput").ap()

    TC = _fixed_tile_context(tile, bass_rust, vector_clock)

    with TC(nc) as tc:
        with tc.tile_pool(name="persist", bufs=1) as per:
            # ---- load inputs ----
            xq_sb = per.tile([128, KTILES * S], bf16, tag="xq")
            xkv_sb = per.tile([128, KTILES * S], bf16, tag="xkv")
            xqb_sb = per.tile([1, S], bf16, tag="xqb")       # ones rows
            xkvb_sb = per.tile([1, S], bf16, tag="xkvb")
            for k in range(KTILES):
                nc.sync.dma_start(xq_sb[:, k * S:(k + 1) * S],
                                  xqT[k * 128:(k + 1) * 128, :])
                nc.sync.dma_start(xkv_sb[:, k * S:(k + 1) * S],
                                  xkvT[k * 128:(k + 1) * 128, :])
            nc.sync.dma_start(xqb_sb[:, :], xqT[D:D + 1, :])
            nc.sync.dma_start(xkvb_sb[:, :], xkvT[D:D + 1, :])

            wq_sb = per.tile([128, KTILES * E], bf16, tag="wq")
            wk_sb = per.tile([128, KTILES * E], bf16, tag="wk")
            wv_sb = per.tile([128, KTILES * E], bf16, tag="wv")
            wqb_sb = per.tile([1, E], bf16, tag="wqb")
            wkb_sb = per.tile([1, E], bf16, tag="wkb")
            wvb_sb = per.tile([1, E], bf16, tag="wvb")
            for w_sb, wb_sb, w_dram in ((wq_sb, wqb_sb, wqT),
                                        (wk_sb, wkb_sb, wkT),
                                        (wv_sb, wvb_sb, wvT)):
                for k in range(KTILES):
                    nc.sync.dma_start(w_sb[:, k * E:(k + 1) * E],
                                      w_dram[k * 128:(k + 1) * 128, :])
                nc.sync.dma_start(wb_sb[:, :], w_dram[D:D + 1, :])

            wo_sb = per.tile([128, 2 * D], bf16, tag="wo")   # pair p at cols p*D
            for p in range(2):
                nc.sync.dma_start(wo_sb[:, p * D:(p + 1) * D],
                                  woT[p * 128:(p + 1) * 128, :])
            cos_sb = per.tile([128, S], bf16, tag="cos")
            sin_sb = per.tile([128, S], bf16, tag="sin")
            nc.sync.dma_start(cos_sb[:, :], cosT[:, :])
            nc.sync.dma_start(sin_sb[:, :], sinT[:, :])
            ones_sb = per.tile([128, 1], bf16, tag="ones")
            nc.sync.dma_start(ones_sb[:, :], ones_col[:, :])

            # persistent activations
            qr_sb = [per.tile([128, S], bf16, tag=f"qr{p}", name=f"qr{p}") for p in range(2)]
            kr_sb = [per.tile([128, S], bf16, tag=f"kr{p}", name=f"kr{p}") for p in range(2)]
            v_sb = per.tile([128, ST * E], bf16, tag="v")    # s-tile st at cols st*E
            ctxn_sb = [per.tile([128, S], bf16, tag=f"ctxn{p}", name=f"ctxn{p}") for p in range(2)]

            # ---- phase A: projections + rope ----
            def qk_projection(w_sb_, wb_sb_, dst, is_q):
                # dst[p][e(128), s] for pair p; rope applied
                with tc.tile_pool(name="qk_ps", bufs=2, space="PSUM") as pps, \
                     tc.tile_pool(name="qk_tmp", bufs=2) as tmp:
                    for p in range(2):
                        q_ps = pps.tile([128, S], f32, tag="q_ps")
                        for sc in range(QC):
                            ss = slice(sc * 512, (sc + 1) * 512)
                            for k in range(KTILES):
                                nc.tensor.matmul(
                                    q_ps[:, ss],
                                    lhsT=w_sb_[:, k * E + p * 128: k * E + (p + 1) * 128],
                                    rhs=(xq_sb if is_q else xkv_sb)[:, k * S + sc * 512:
                                                                    k * S + (sc + 1) * 512],
                                    start=(k == 0), stop=False)
                            nc.tensor.matmul(
                                q_ps[:, ss],
                                lhsT=wb_sb_[:, p * 128:(p + 1) * 128],
                                rhs=(xqb_sb if is_q else xkvb_sb)[:, ss],
                                start=False, stop=True)
                        qb = tmp.tile([128, S], bf16, tag="qb")
                        qsw = tmp.tile([128, S], bf16, tag="qsw")
                        qcos = tmp.tile([128, S], bf16, tag="qcos")
                        nc.scalar.activation(qb[:, :], q_ps[:, :], Copy)
                        # swap halves within each 64-block (partition swap, DMA)
                        for a, bdst in ((0, 32), (32, 0), (64, 96), (96, 64)):
                            nc.sync.dma_start(qsw[bdst:bdst + 32, :], qb[a:a + 32, :])
                        nc.vector.tensor_tensor(qcos[:, :], qb[:, :], cos_sb[:, :], mult)
                        nc.vector.tensor_tensor(qsw[:, :], qsw[:, :], sin_sb[:, :], mult)
                        nc.vector.tensor_tensor(dst[p][:, :], qcos[:, :], qsw[:, :], add)

            qk_projection(wq_sb, wqb_sb, qr_sb, True)
            qk_projection(wk_sb, wkb_sb, kr_sb, False)

            with tc.tile_pool(name="v_ps", bufs=2, space="PSUM") as vps:
                for st in range(ST):
                    v_ps = vps.tile([128, E], f32, tag="v_ps")
                    for k in range(KTILES):
                        nc.tensor.matmul(
                            v_ps[:, :],
                            lhsT=xkv_sb[:, k * S + st * 128: k * S + (st + 1) * 128],
                            rhs=wv_sb[:, k * E:(k + 1) * E],
                            start=(k == 0), stop=False)
                    nc.tensor.matmul(
                        v_ps[:, :],
                        lhsT=xkvb_sb[:, st * 128:(st + 1) * 128],
                        rhs=wvb_sb[:, :],
                        start=False, stop=True)
                    nc.scalar.activation(v_sb[:, st * E:(st + 1) * E], v_ps[:, :], Copy)

            # ---- phase B: SDPA ----
            with tc.tile_pool(name="sc_ps", bufs=2, space="PSUM") as scp, \
                 tc.tile_pool(name="cd_ps", bufs=1, space="PSUM") as cdp, \
                 tc.tile_pool(name="e_sb", bufs=3) as esp, \
                 tc.tile_pool(name="norm", bufs=2) as nrm, \
                 tc.tile_pool(name="ldram", bufs=2, space="DRAM") as ldr:
                for qh in range(QC):
                    qs = slice(qh * 512, (qh + 1) * 512)
                    ctx_ps = [cdp.tile([128, 512], f32, tag=f"ctx{p}", name=f"ctx{p}") for p in range(2)]
                    den_ps = cdp.tile([128, 512], f32, tag="den")
                    nc.vector.memset(den_ps[:, :], 1.0)
                    e_tiles = [None, None]
                    for ki in range(ST):
                        ks = slice(ki * 128, (ki + 1) * 128)
                        for p in range(2):
                            s_ps = scp.tile([128, 1024], f32, tag="s")
                            nc.tensor.matmul(
                                s_ps[:, 0:512],
                                lhsT=kr_sb[p][0:64, ks], rhs=qr_sb[p][0:64, qs],
                                tile_position=(0, 0), start=True, stop=True)
                            nc.tensor.matmul(
                                s_ps[:, 512:1024],
                                lhsT=kr_sb[p][64:128, ks], rhs=qr_sb[p][64:128, qs],
                                tile_position=(64, 0), start=True, stop=True)
                            e_sb = esp.tile([128, 1024], bf16, tag=f"e{p}")
                            nc.scalar.activation(e_sb[:, :], s_ps[:, :], Exp)
                            e_tiles[p] = e_sb
                            nc.tensor.matmul(
                                ctx_ps[p][0:64, :],
                                lhsT=v_sb[:, ki * E + (2 * p) * 64: ki * E + (2 * p) * 64 + 64],
                                rhs=e_sb[:, 0:512],
                                tile_position=(0, 0),
                                start=(ki == 0), stop=(ki == ST - 1),
                                skip_group_check=True)
                            nc.tensor.matmul(
                                ctx_ps[p][64:128, :],
                                lhsT=v_sb[:, ki * E + (2 * p + 1) * 64: ki * E + (2 * p + 1) * 64 + 64],
                                rhs=e_sb[:, 512:1024],
                                tile_position=(0, 64),
                                start=(ki == 0), stop=(ki == ST - 1),
                                skip_group_check=True)
                        # denominators: 4 heads, one col group each
                        for g, (p, half) in enumerate(((0, 0), (0, 1), (1, 0), (1, 1))):
                            nc.tensor.matmul(
                                den_ps[g * 32: g * 32 + 1, :],
                                lhsT=ones_sb[:, :],
                                rhs=e_tiles[p][:, half * 512:(half + 1) * 512],
                                tile_position=(0, g * 32),
                                start=(ki == 0), stop=(ki == ST - 1),
                                skip_group_check=True)
                    # normalize: linv rows -> DRAM roundtrip broadcast -> ctx * linv
                    linv = nrm.tile([128, 512], f32, tag="linv")
                    nc.vector.reciprocal(linv[:, :], den_ps[:, :])
                    lscr = ldr.tile([4, 512], f32, tag="lscr")
                    nc.sync.dma_start(
                        lscr[:, :], linv[0:128:32, :])
                    lbc = [nrm.tile([128, 512], f32, tag=f"lbc{p}", name=f"lbc{p}") for p in range(2)]
                    for g, (p, half) in enumerate(((0, 0), (0, 1), (1, 0), (1, 1))):
                        nc.sync.dma_start(
                            lbc[p][half * 64:(half + 1) * 64, :],
                            lscr[g:g + 1, :].partition_broadcast(64))
                    for p in range(2):
                        nc.vector.tensor_tensor(
                            ctxn_sb[p][:, qs], ctx_ps[p][:, :], lbc[p][:, :], mult)

            # ---- phase C: output projection ----
            with tc.tile_pool(name="o_ps", bufs=2, space="PSUM") as ops, \
                 tc.tile_pool(name="o_sb", bufs=2) as osb:
                for st in range(ST):
                    o_ps = ops.tile([128, D], f32, tag="o")
                    for ch in range(2):
                        cs = slice(ch * 512, (ch + 1) * 512)
                        for p in range(2):
                            nc.tensor.matmul(
                                o_ps[:, cs],
                                lhsT=ctxn_sb[p][:, st * 128:(st + 1) * 128],
                                rhs=wo_sb[:, p * D + ch * 512: p * D + (ch + 1) * 512],
                                start=(p == 0), stop=(p == 1))
                    o_out = osb.tile([128, D], f32, tag="oo")
                    nc.vector.tensor_copy(o_out[:, :], o_ps[:, :])
                    nc.sync.dma_start(out[st * 128:(st + 1) * 128, :], o_out[:, :])

    if split_waits:
        _split_multiwait_instructions(nc, mybir, bass_rust.SyncInfo)
    return nc


def kernel(x_q, x_kv, wq, bq, wk, bk, wv, bv, wo, bo):
    from concourse import bass_utils

    x_q = np.asarray(x_q, dtype=np.float32)
    x_kv = np.asarray(x_kv, dtype=np.float32)
    wq = np.asarray(wq, dtype=np.float32); bq = np.asarray(bq, dtype=np.float32)
    wk = np.asarray(wk, dtype=np.float32); bk = np.asarray(bk, dtype=np.float32)
    wv = np.asarray(wv, dtype=np.float32); bv = np.asarray(bv, dtype=np.float32)
    wo = np.asarray(wo, dtype=np.float32); bo = np.asarray(bo, dtype=np.float32)

    in_maps = _host_prep(x_q, x_kv, wq, bq, wk, bk, wv, bv, wo)

    if "prog" not in _PROGRAM_CACHE:
        _PROGRAM_CACHE["prog"] = build_program()
    nc = _PROGRAM_CACHE["prog"]

    res = bass_utils.run_bass_kernel_spmd(
        nc, in_maps, core_ids=list(range(N_CORES)),
        trace=os.environ.get("KERNEL_TRACE", "") == "1")
    _PROGRAM_CACHE["last_result"] = res

    out = np.zeros((B, S, D), dtype=np.float32)
    for c in range(N_CORES):
        out[c // QUADS] += res.results[c]["out"]
    out += bo[None, None, :]
    return out



# revision 7
# speedup vs baseline: 1.3705x; 1.3705x over previous
"""Trainium2 Bass kernel for nn_CrossAttentionBlock (B=2, S=2048, D=1024, H=16, HD=64).

Sharding: 8 cores = 2 batches x 4 head-quads (4 heads each, E=256 channels).
Each core computes q/k/v projections for its quad, RoPE, SDPA, and a partial
output projection [S, D]; host sums the 4 partials per batch and adds bo.

v2 pipeline (vs the phase-serial baseline):
  - The SDPA inner loop is software-pipelined: scores(ki+1) is emitted before
    ctx(ki) so the PE never head-of-line blocks on the ScalarE exp.  ScalarE
    does nothing but exp (the hard ~147us floor); all PSUM->SBUF copies are on
    DVE, and the projections / output projection are interleaved into the
    exp-bound window as PE filler work.
  - Pair-major ki loop: pair 0's exp tiles are kept in a persistent SBUF ring
    so softmax denominators run as 4-way col-tiled concurrent matmuls in pair
    1's loop.  PSUM: scores ping-pong 4 banks + ctx 2 + den 1 + aux 1 = 8.
  - Inputs are host-packed so each s-chunk is a single DMA; the DMA order is
    prioritized so scores/exp start ~17us in instead of ~40us.
  - RoPE uses a host-permuted sin table: u = q*sinP read straight from PSUM,
    partition-swap of u via SBUF-SBUF DMA, one DVE add.  No ScalarE copies.
"""
import collections
import os
import sys

sys.path.insert(0, "/opt/trn_rl_repo")

import numpy as np
import ml_dtypes

BF16 = ml_dtypes.bfloat16

B, S, D, H = 2, 2048, 1024, 16
HD = D // H          # 64
DIM = HD // 2        # 32
QUADS = 4
E = D // QUADS       # 256 channels per core
ROPE_BASE = 10000.0
N_CORES = 8

KT = D // 128        # 8 d k-tiles
ST = S // 128        # 16 s-tiles
QC = S // 512        # 4 s-chunks
CW = KT * 512        # 4096 packed x cols per s-chunk


def _pack_x(xT):
    """[D(+1), S] d-major -> [128, QC*CW] s-chunk-major packed layout.

    packed[p, sc*CW + k*512 + s] = xT[k*128 + p, sc*512 + s]
    (bias row, if present, is returned separately)
    """
    xmain = xT[:D].reshape(KT, 128, QC, 512)
    packed = xmain.transpose(1, 2, 0, 3).reshape(128, QC * CW)
    return np.ascontiguousarray(packed)


def _pack_w(wT):
    """[D(+1), E] -> [128, KT*E]: packed[p, k*E + c] = wT[k*128 + p, c]."""
    return np.ascontiguousarray(
        wT[:D].reshape(KT, 128, E).transpose(1, 0, 2).reshape(128, KT * E))


def _host_prep(x_q, x_kv, wq, bq, wk, bk, wv, bv, wo, with_bias):
    perm = np.concatenate([np.arange(0, HD, 2), np.arange(1, HD, 2)])  # even|odd
    scale = 1.0 / np.sqrt(HD)

    freqs = np.exp(-np.arange(DIM, dtype=np.float64) * np.log(ROPE_BASE) / DIM)
    ang = np.arange(S, dtype=np.float64)[:, None] * freqs[None, :]     # [S, 32]
    cos = np.cos(ang).T                                                # [32, S]
    sin = np.sin(ang).T
    # rot-half: dst = q*cos64 + swap(q)*sin64, cos64=[cos;cos], sin64=[-sin;sin]
    # permuted-sin trick: ship sinP = swap_rows(sin64) = [sin;-sin]; then
    # u = q*sinP and swap(u) = swap(q)*sin64 exactly.
    cos64 = np.concatenate([cos, cos], axis=0)                         # [64, S]
    sinp64 = np.concatenate([sin, -sin], axis=0)
    cosT = np.concatenate([cos64, cos64], axis=0).astype(BF16)         # [128, S]
    sinPT = np.concatenate([sinp64, sinp64], axis=0).astype(BF16)

    def proj_mat(w, b, permute, s):
        blocks, brows = [], []
        for h in range(4):
            rows = slice(h * HD, (h + 1) * HD)
            wb_ = w[rows, :]
            bb = b[rows]
            if permute:
                wb_ = wb_[perm, :]
                bb = bb[perm]
            blocks.append(wb_ * s)
            brows.append(bb * s)
        wstack = np.concatenate(blocks, axis=0)          # [256, 1024]
        bstack = np.concatenate(brows, axis=0)           # [256]
        return wstack.T, bstack                          # [1024, 256], [256]

    ones_row = np.ones((1, S), dtype=BF16)
    in_maps = []
    for c in range(N_CORES):
        b_ = c // QUADS
        g = c % QUADS
        hs = slice(g * E, (g + 1) * E)
        wqT_, bq_ = proj_mat(wq[hs, :], bq[hs], True, scale)
        wkT_, bk_ = proj_mat(wk[hs, :], bk[hs], True, 1.0)
        wvT_, bv_ = proj_mat(wv[hs, :], bv[hs], False, 1.0)
        m = {
            "xqT": _pack_x(x_q[b_].T.astype(BF16)),
            "xkvT": _pack_x(x_kv[b_].T.astype(BF16)),
            "wqT": _pack_w(wqT_.astype(BF16)),
            "wkT": _pack_w(wkT_.astype(BF16)),
            "wvT": _pack_w(wvT_.astype(BF16)),
            "woT": np.ascontiguousarray(
                wo[:, hs].T.reshape(2, 128, D).transpose(1, 0, 2).reshape(128, 2 * D)
            ).astype(BF16),
            "cosT": np.ascontiguousarray(cosT),
            "sinPT": np.ascontiguousarray(sinPT),
            "ones_col": np.ones((128, 1), dtype=BF16),
        }
        if with_bias:
            m["xqb"] = ones_row.copy()
            m["xkvb"] = ones_row.copy()
            m["wqb"] = bq_[None, :].astype(BF16)
            m["wkb"] = bk_[None, :].astype(BF16)
            m["wvb"] = bv_[None, :].astype(BF16)
        in_maps.append(m)
    return in_maps


# ---------------------------------------------------------------------------
_PROGRAM_CACHE = {}


def _fixed_tile_context(tile_mod, bass_rust_mod, vector_clock_mod):
    """TileContext whose tail drain splits multi-sem waits into single-wait
    NOPs (this walrus rejects >1 sync-wait on one instruction)."""
    SyncInfo = bass_rust_mod.SyncInfo
    ScopedClock = vector_clock_mod.ScopedClock

    class TC(tile_mod.TileContext):
        def _drain_and_barrier(self, tick_clock, wait_clock):
            harvest = self.nc.sync.nop(nofuse=True)
            wait_clock.add_sem_waits(
                harvest.ins, ScopedClock({None: tick_clock.global_clock}))
            si = harvest.ins.sync_info
            waits = list(si.on_wait) if si is not None else []
            if len(waits) > 1:
                harvest.ins.sync_info = SyncInfo(
                    on_wait=[waits[0]], on_update=list(si.on_update))
                for w in waits[1:]:
                    nop = self.nc.sync.nop(nofuse=True)
                    nop.ins.sync_info = SyncInfo(on_wait=[w], on_update=[])
            self.nc.sync.drain()
            self.nc.all_engine_barrier()
            assert self.sems is not None
            popped = self.nc._tile_sem_poison_stack.pop()
            assert popped is self._sem_poison
            self.nc.clear_and_free_semaphores(list(self.sems.allocated().values()))
            self.nc.all_engine_barrier()

    return TC


def _split_multiwait_instructions(nc, mybir, SyncInfo):
    """This walrus build rejects >1 sync-wait per instruction; hoist extra
    waits onto single-wait NOPs inserted just before, on the same engine."""
    ctr = 0
    for blk in nc.m.functions[0].blocks:
        insts = blk.instructions
        i = 0
        while i < len(insts):
            inst = insts[i]
            si = inst.sync_info
            if si is not None and len(si.on_wait) > 1:
                waits = list(si.on_wait)
                inst.sync_info = SyncInfo(on_wait=[waits[-1]],
                                          on_update=list(si.on_update))
                nops = []
                for w in waits[:-1]:
                    nop = mybir.InstNoOp(name=f"waitsplit_{ctr}", ins=[], outs=[])
                    ctr += 1
                    nop.engine = inst.engine
                    nop.sync_info = SyncInfo(on_wait=[w], on_update=[])
                    nops.append(nop)
                insts[i:i] = nops
                i += len(nops)
            i += 1
    return ctr


def build_program(with_bias=False):
    import concourse.bass as bass
    import concourse.mybir as mybir
    import concourse.tile as tile
    import bass_rust
    from concourse import vector_clock

    f32 = mybir.dt.float32
    bf16 = mybir.dt.bfloat16
    Exp = mybir.ActivationFunctionType.Exp
    mult = mybir.AluOpType.mult
    add = mybir.AluOpType.add

    nc = bass.Bass("TRN2", target_bir_lowering=False, debug=False,
                   num_devices=N_CORES)

    xqT = nc.dram_tensor("xqT", [128, QC * CW], bf16, kind="ExternalInput").ap()
    xkvT = nc.dram_tensor("xkvT", [128, QC * CW], bf16, kind="ExternalInput").ap()
    wqT = nc.dram_tensor("wqT", [128, KT * E], bf16, kind="ExternalInput").ap()
    wkT = nc.dram_tensor("wkT", [128, KT * E], bf16, kind="ExternalInput").ap()
    wvT = nc.dram_tensor("wvT", [128, KT * E], bf16, kind="ExternalInput").ap()
    woT = nc.dram_tensor("woT", [128, 2 * D], bf16, kind="ExternalInput").ap()
    cosT = nc.dram_tensor("cosT", [128, S], bf16, kind="ExternalInput").ap()
    sinPT = nc.dram_tensor("sinPT", [128, S], bf16, kind="ExternalInput").ap()
    ones_col = nc.dram_tensor("ones_col", [128, 1], bf16, kind="ExternalInput").ap()
    if with_bias:
        xqb = nc.dram_tensor("xqb", [1, S], bf16, kind="ExternalInput").ap()
        xkvb = nc.dram_tensor("xkvb", [1, S], bf16, kind="ExternalInput").ap()
        wqb = nc.dram_tensor("wqb", [1, E], bf16, kind="ExternalInput").ap()
        wkb = nc.dram_tensor("wkb", [1, E], bf16, kind="ExternalInput").ap()
        wvb = nc.dram_tensor("wvb", [1, E], bf16, kind="ExternalInput").ap()
    out = nc.dram_tensor("out", [S, D], f32, kind="ExternalOutput").ap()

    TC = _fixed_tile_context(tile, bass_rust, vector_clock)

    with TC(nc) as tc:
        with tc.tile_pool(name="per", bufs=1) as per, \
             tc.tile_pool(name="rp", bufs=2) as rp, \
             tc.tile_pool(name="esb", bufs=3) as esp, \
             tc.tile_pool(name="osb", bufs=2) as osb, \
             tc.tile_pool(name="nrm", bufs=2) as nrm:

            # ---- persistent tiles ----
            xq_sb = per.tile([128, QC * CW], bf16, tag="xq", name="xq")
            xkv_sb = per.tile([128, QC * CW], bf16, tag="xkv", name="xkv")
            wq_sb = per.tile([128, KT * E], bf16, tag="wq", name="wq")
            wk_sb = per.tile([128, KT * E], bf16, tag="wk", name="wk")
            wv_sb = per.tile([128, KT * E], bf16, tag="wv", name="wv")
            wo_sb = per.tile([128, 2 * D], bf16, tag="wo", name="wo")
            cos_sb = per.tile([128, S], bf16, tag="cos", name="cos")
            sinp_sb = per.tile([128, S], bf16, tag="sinp", name="sinp")
            ones_sb = per.tile([128, 1], bf16, tag="ones", name="ones")
            qr_sb = [per.tile([128, S], bf16, tag=f"qr{p}", name=f"qr{p}")
                     for p in range(2)]
            kr_sb = [per.tile([128, S], bf16, tag=f"kr{p}", name=f"kr{p}")
                     for p in range(2)]
            v_sb = per.tile([128, ST * E], bf16, tag="v", name="v")
            ctxn_sb = [per.tile([128, S], bf16, tag=f"ctxn{p}", name=f"ctxn{p}")
                       for p in range(2)]
            # pair-0 exp tiles for one qh, kept so dens run in pair 1's loop
            e0_sb = per.tile([128, ST * 1024], bf16, tag="e0", name="e0")
            ew_sb = per.tile([1, 1], f32, tag="ew", name="ew")
            if with_bias:
                xqb_sb = per.tile([1, S], bf16, tag="xqb", name="xqb")
                xkvb_sb = per.tile([1, S], bf16, tag="xkvb", name="xkvb")
                wqb_sb = per.tile([1, E], bf16, tag="wqb", name="wqb")
                wkb_sb = per.tile([1, E], bf16, tag="wkb", name="wkb")
                wvb_sb = per.tile([1, E], bf16, tag="wvb", name="wvb")

            # ---- input DMAs, priority order ----
            nc.sync.dma_start(ones_sb[:, :], ones_col[:, :])
            nc.sync.dma_start(cos_sb[:, :], cosT[:, :])
            nc.sync.dma_start(sinp_sb[:, :], sinPT[:, :])
            if with_bias:
                nc.sync.dma_start(xkvb_sb[:, :], xkvb[:, :])
                nc.sync.dma_start(xqb_sb[:, :], xqb[:, :])
                nc.sync.dma_start(wkb_sb[:, :], wkb[:, :])
                nc.sync.dma_start(wqb_sb[:, :], wqb[:, :])
                nc.sync.dma_start(wvb_sb[:, :], wvb[:, :])
            nc.sync.dma_start(wk_sb[:, :], wkT[:, :])
            nc.sync.dma_start(xkv_sb[:, 0:CW], xkvT[:, 0:CW])
            nc.sync.dma_start(wq_sb[:, :], wqT[:, :])
            nc.sync.dma_start(xq_sb[:, 0:CW], xqT[:, 0:CW])
            nc.sync.dma_start(wv_sb[:, :], wvT[:, :])
            for sc in range(1, QC):
                nc.sync.dma_start(xkv_sb[:, sc * CW:(sc + 1) * CW],
                                  xkvT[:, sc * CW:(sc + 1) * CW])
            for sc in range(1, QC):
                nc.sync.dma_start(xq_sb[:, sc * CW:(sc + 1) * CW],
                                  xqT[:, sc * CW:(sc + 1) * CW])
            nc.sync.dma_start(wo_sb[:, :], woT[:, :])

            # preload the exp table set during the DMA window
            nc.scalar.activation(ew_sb[0:1, 0:1], ones_sb[0:1, 0:1], Exp)

            # ---- helpers ----
            def rope_chunk(q_ps, dst, sc):
                scs = slice(sc * 512, (sc + 1) * 512)
                u = rp.tile([128, 512], bf16, tag="u", name="u")
                a = rp.tile([128, 512], bf16, tag="a", name="a")
                usw = rp.tile([128, 512], bf16, tag="usw", name="usw")
                nc.vector.tensor_tensor(u[:, :], q_ps[:, :], sinp_sb[:, scs], mult)
                nc.vector.tensor_tensor(a[:, :], q_ps[:, :], cos_sb[:, scs], mult)
                for s0, d0 in ((0, 32), (32, 0), (64, 96), (96, 64)):
                    nc.sync.dma_start(usw[d0:d0 + 32, :], u[s0:s0 + 32, :])
                nc.vector.tensor_tensor(dst[:, scs], a[:, :], usw[:, :], add)

            def proj_chunk(pool, tag, which, p, sc):
                if which == "q":
                    w_, wb_, srcb_, dst = wq_sb, None, None, qr_sb
                    if with_bias:
                        wb_, srcb_ = wqb_sb, xqb_sb
                    src = xq_sb
                else:
                    w_, wb_, srcb_, dst = wk_sb, None, None, kr_sb
                    if with_bias:
                        wb_, srcb_ = wkb_sb, xkvb_sb
                    src = xkv_sb
                q_ps = pool.tile([128, 512], f32, tag=tag, name=f"{which}proj")
                for k in range(KT):
                    for h2 in range(2):
                        co = k * E + p * 128 + h2 * 64
                        nc.tensor.matmul(
                            q_ps[h2 * 64:(h2 + 1) * 64, :],
                            lhsT=w_[:, co:co + 64],
                            rhs=src[:, sc * CW + k * 512: sc * CW + (k + 1) * 512],
                            tile_position=(0, h2 * 64),
                            start=(k == 0),
                            stop=(k == KT - 1) and not with_bias,
                            skip_group_check=True)
                if with_bias:
                    nc.tensor.matmul(
                        q_ps[:, :], lhsT=wb_[:, p * 128:(p + 1) * 128],
                        rhs=srcb_[:, sc * 512:(sc + 1) * 512],
                        start=False, stop=True, skip_group_check=True)
                rope_chunk(q_ps, dst[p], sc)

            def vproj_st(pool, tag, st):
                sc, si = st // 4, st % 4
                v_ps = pool.tile([128, E], f32, tag=tag, name="vproj")
                for k in range(KT):
                    for h2 in range(2):
                        co = sc * CW + k * 512 + si * 128 + h2 * 64
                        nc.tensor.matmul(
                            v_ps[h2 * 64:(h2 + 1) * 64, :],
                            lhsT=xkv_sb[:, co:co + 64],
                            rhs=wv_sb[:, k * E:(k + 1) * E],
                            tile_position=(0, h2 * 64),
                            start=(k == 0),
                            stop=(k == KT - 1) and not with_bias,
                            skip_group_check=True)
                if with_bias:
                    nc.tensor.matmul(
                        v_ps[:, :],
                        lhsT=xkvb_sb[:, st * 128:(st + 1) * 128],
                        rhs=wvb_sb[:, :],
                        start=False, stop=True, skip_group_check=True)
                nc.vector.tensor_copy(v_sb[:, st * E:(st + 1) * E], v_ps[:, :])

            def outproj_chunk(pool, tag, st, dch):
                o_ps = pool.tile([128, 512], f32, tag=tag, name="oproj")
                for p in range(2):
                    for h2 in range(2):
                        nc.tensor.matmul(
                            o_ps[h2 * 64:(h2 + 1) * 64, :],
                            lhsT=ctxn_sb[p][:, st * 128 + h2 * 64:
                                            st * 128 + h2 * 64 + 64],
                            rhs=wo_sb[:, p * D + dch * 512: p * D + (dch + 1) * 512],
                            tile_position=(0, h2 * 64),
                            start=(p == 0), stop=(p == 1), skip_group_check=True)
                o_t = osb.tile([128, 512], f32, tag="o", name="o")
                nc.vector.tensor_copy(o_t[:, :], o_ps[:, :])
                nc.sync.dma_start(
                    out[st * 128:(st + 1) * 128, dch * 512:(dch + 1) * 512],
                    o_t[:, :])

            # ---- lead-in (own PSUM pool, released before SDPA) ----
            with tc.tile_pool(name="pps", bufs=2, space="PSUM") as pps:
                wps = pps.tile([128, 512], f32, tag="warm", name="warm", bufs=1)
                for _ in range(12):
                    nc.tensor.matmul(wps[:, :], lhsT=cos_sb[:, 0:128],
                                     rhs=cos_sb[:, 0:512], start=True, stop=True)
                for p in range(2):
                    proj_chunk(pps, "pp", "k", p, 0)
                for p in range(2):
                    proj_chunk(pps, "pp", "q", p, 0)
                vproj_st(pps, "pp", 0)
                vproj_st(pps, "pp", 1)

            # ---- SDPA + interleaved fillers ----
            with tc.tile_pool(name="scp", bufs=2, space="PSUM") as scp, \
                 tc.tile_pool(name="cdp", bufs=1, space="PSUM") as cdp, \
                 tc.tile_pool(name="dnp", bufs=1, space="PSUM") as dnp, \
                 tc.tile_pool(name="axp", bufs=1, space="PSUM") as axp, \
                 tc.tile_pool(name="ldr", bufs=2, space="DRAM") as ldr:

                fill = collections.defaultdict(list)
                # qh0/p0: v-projections (ctx(qh0,ki) needs v(st=ki)); kproj jit
                for ki in range(14):
                    fill[(0, 0, ki)].append(
                        lambda st=ki + 2: vproj_st(axp, "aux", st))
                for i, step in enumerate((1, 2, 5, 6, 9, 10)):
                    sc, p = 1 + i // 2, i % 2
                    fill[(0, 0, step)].append(
                        lambda sc=sc, p=p: proj_chunk(axp, "aux", "k", p, sc))
                # q-projection chunk qc feeds qh=qc; emit during qh=qc-1
                for qh in range(3):
                    fill[(qh, 1, 6)].append(
                        lambda qc=qh + 1: proj_chunk(axp, "aux", "q", 0, qc))
                    fill[(qh, 1, 10)].append(
                        lambda qc=qh + 1: proj_chunk(axp, "aux", "q", 1, qc))
                # out-projection of qh-1 spread across qh's two sub-loops
                for qh in range(1, QC):
                    for i, step in enumerate((2, 5, 8, 11)):
                        st0, dch0 = (qh - 1) * 4 + i // 2, i % 2
                        fill[(qh, 0, step)].append(
                            lambda st=st0, dch=dch0:
                                outproj_chunk(axp, "aux", st, dch))
                        st1, dch1 = (qh - 1) * 4 + (4 + i) // 2, i % 2
                        fill[(qh, 1, step)].append(
                            lambda st=st1, dch=dch1:
                                outproj_chunk(axp, "aux", st, dch))

                for qh in range(QC):
                    qs = slice(qh * 512, (qh + 1) * 512)
                    ctx_ps = [cdp.tile([128, 512], f32, tag=f"ctx{p}",
                                       name=f"ctx{p}") for p in range(2)]
                    den_ps = dnp.tile([128, 512], f32, tag="den", name="den")
                    # unwritten partitions must stay finite for the full-width
                    # reciprocal below (only rows 0,32,64,96 are consumed)
                    nc.vector.memset(den_ps[:, :], 1.0)
                    e1_tiles = {}

                    def scores_exp(p, ki):
                        s_ps = scp.tile([128, 1024], f32, tag="s", name="s")
                        nc.tensor.matmul(
                            s_ps[:, 0:512],
                            lhsT=kr_sb[p][0:64, ki * 128:(ki + 1) * 128],
                            rhs=qr_sb[p][0:64, qs],
                            tile_position=(0, 0), start=True, stop=True)
                        nc.tensor.matmul(
                            s_ps[:, 512:1024],
                            lhsT=kr_sb[p][64:128, ki * 128:(ki + 1) * 128],
                            rhs=qr_sb[p][64:128, qs],
                            tile_position=(64, 0), start=True, stop=True)
                        if p == 0:
                            e_t = e0_sb[:, ki * 1024:(ki + 1) * 1024]
                        else:
                            e_t = esp.tile([128, 1024], bf16, tag="e1", name="e1")
                            e1_tiles[ki] = e_t
                        nc.scalar.activation(e_t, s_ps[:, :], Exp)
                        return e_t

                    for p in range(2):
                        e_cur = scores_exp(p, 0)
                        for ki in range(ST):
                            e_this = e_cur
                            if ki < ST - 1:
                                e_cur = scores_exp(p, ki + 1)
                            for h2 in range(2):
                                vo = ki * E + (2 * p + h2) * 64
                                nc.tensor.matmul(
                                    ctx_ps[p][h2 * 64:(h2 + 1) * 64, :],
                                    lhsT=v_sb[:, vo:vo + 64],
                                    rhs=e_this[:, h2 * 512:(h2 + 1) * 512],
                                    tile_position=(0, h2 * 64),
                                    start=(ki == 0), stop=(ki == ST - 1),
                                    skip_group_check=True)
                            if p == 1:
                                for g, (pp_, half) in enumerate(
                                        ((0, 0), (0, 1), (1, 0), (1, 1))):
                                    src = (e0_sb[:, ki * 1024:(ki + 1) * 1024]
                                           if pp_ == 0 else e_this)
                                    nc.tensor.matmul(
                                        den_ps[g * 32:g * 32 + 1, :],
                                        lhsT=ones_sb[:, :],
                                        rhs=src[:, half * 512:(half + 1) * 512],
                                        tile_position=(0, g * 32),
                                        start=(ki == 0), stop=(ki == ST - 1),
                                        skip_group_check=True)
                            for f in fill.get((qh, p, ki), []):
                                f()

                    # normalize: linv rows -> partition broadcast -> ctx * linv
                    linv = nrm.tile([128, 512], f32, tag="linv", name="linv")
                    nc.vector.reciprocal(linv[:, :], den_ps[:, :])
                    lscr = ldr.tile([4, 512], f32, tag="lscr", name="lscr")
                    nc.sync.dma_start(lscr[:, :], linv[0:128:32, :])
                    lbc = [nrm.tile([128, 512], f32, tag=f"lbc{p}",
                                    name=f"lbc{p}") for p in range(2)]
                    for g, (p, half) in enumerate(((0, 0), (0, 1), (1, 0), (1, 1))):
                        nc.sync.dma_start(
                            lbc[p][half * 64:(half + 1) * 64, :],
                            lscr[g:g + 1, :].partition_broadcast(64))
                    for p in range(2):
                        nc.vector.tensor_tensor(
                            ctxn_sb[p][:, qs], ctx_ps[p][:, :], lbc[p][:, :], mult)

                # epilogue: last qh's out-projection, ping-pong on scores slots
                for c in range(8):
                    outproj_chunk(scp, "s", 12 + c // 2, c % 2)

    _split_multiwait_instructions(nc, mybir, bass_rust.SyncInfo)
    return nc


def kernel(x_q, x_kv, wq, bq, wk, bk, wv, bv, wo, bo):
    from concourse import bass_utils

    x_q = np.asarray(x_q, dtype=np.float32)
    x_kv = np.asarray(x_kv, dtype=np.float32)
    wq = np.asarray(wq, dtype=np.float32); bq = np.asarray(bq, dtype=np.float32)
    wk = np.asarray(wk, dtype=np.float32); bk = np.asarray(bk, dtype=np.float32)
    wv = np.asarray(wv, dtype=np.float32); bv = np.asarray(bv, dtype=np.float32)
    wo = np.asarray(wo, dtype=np.float32); bo = np.asarray(bo, dtype=np.float32)

    with_bias = bool(bq.any() or bk.any() or bv.any())
    in_maps = _host_prep(x_q, x_kv, wq, bq, wk, bk, wv, bv, wo, with_bias)

    key = f"prog_{with_bias}"
    if key not in _PROGRAM_CACHE:
        _PROGRAM_CACHE[key] = build_program(with_bias)
    nc = _PROGRAM_CACHE[key]

    res = bass_utils.run_bass_kernel_spmd(
        nc, in_maps, core_ids=list(range(N_CORES)),
        trace=os.environ.get("KERNEL_TRACE", "") == "1")
    _PROGRAM_CACHE["last_result"] = res

    outp = np.zeros((B, S, D), dtype=np.float32)
    for c in range(N_CORES):
        outp[c // QUADS] += res.results[c]["out"]
    if bo.any():
        outp += bo[None, None, :]
    return outp
